# revision 27
# baseline (speedup 1.0000x reference)
"""Position-routed MLP (expert = position % 8) on 8 NeuronCores.

Expert-parallel: core e runs expert e's dense MLP on its 2048 tokens.
Host gathers tokens by expert (stable sort, same as the reference),
transposes activations to [feature, token] so every matmul's
contraction dim sits on SBUF partitions with no on-device transposes:

    guT[j, t]  = sum_h W1[h, j] * xT[h, t]      (fp32r matmuls)
    interT     = silu(gateT) * upT              (ACT + DVE)
    outT[o, t] = sum_i W2[i, o] * interT[i, t]  (fp32r matmuls)

Tokens are processed in 2 halves of 1024 so x-half + inter + streamed
weight tiles fit in SBUF; weights are re-read once per half.

Weights are pre-tiled on the host ([tile, partition, free] contiguous)
so each weight tile is a single full-bandwidth DMA. Weight loads go on
the scalar engine's HWDGE ring, x loads on sync's, stores on gpsimd
(SWDGE) to avoid FIFO head-of-line blocking between streams.
"""

import numpy as np

import concourse.bass as bass
import concourse.tile as tile
from concourse.bass import _add_dep_helper
from concourse import bacc, mybir
from concourse.bass_utils import run_bass_kernel_spmd

E = 8
H = 2048
EI = 1024
TPC = 4 * 4096 // E  # tokens per core = 2048
P = 128
KO = H // P    # 16 contraction subtiles for stage 1
IO = EI // P   # 8 contraction subtiles for stage 2
HO = H // P    # 16 output-row tiles for stage 2
NT = 512       # moving free dim per matmul (fp32 max)
HALVES = 2
THALF = TPC // HALVES  # 1024
TT = THALF // NT       # 2

F32 = mybir.dt.float32
F32R = mybir.dt.float32r


def _build_mlp(nc: bass.Bass, tc: tile.TileContext, xT, w1g, w1u, w2, outT, ctx):
    # fp32r tags everywhere a tensor feeds a matmul: the BIR verifier
    # requires producers of fp32r-matmul operands to be fp32r themselves.
    xv = xT.bitcast(F32R).rearrange("(ko p) t -> p ko t", p=P)   # [128, 16, 2048]
    w1gv = w1g.bitcast(F32R)  # [IO, 128, KO*128] pre-tiled on host
    w1uv = w1u.bitcast(F32R)  # [IO, 128, KO*128]
    w2v = w2.bitcast(F32R)    # [HO, 128, IO*128]
    ov = outT.rearrange("(ho p) t -> p ho t", p=P)               # [128, 16, 2048]

    xpool = ctx.enter_context(tc.tile_pool(name="x", bufs=1))
    ipool = ctx.enter_context(tc.tile_pool(name="inter", bufs=1))
    wgpool = ctx.enter_context(tc.tile_pool(name="wg", bufs=2))
    wupool = ctx.enter_context(tc.tile_pool(name="wu", bufs=2))
    w2pool = ctx.enter_context(tc.tile_pool(name="w2", bufs=7))
    tmppool = ctx.enter_context(tc.tile_pool(name="tmp", bufs=3))
    opool = ctx.enter_context(tc.tile_pool(name="ostage", bufs=3))
    psum1 = ctx.enter_context(tc.tile_pool(name="psum1", bufs=6, space="PSUM"))
    psum2 = ctx.enter_context(tc.tile_pool(name="psum2", bufs=2, space="PSUM"))

    last_s2_mm = None  # last stage-2 matmul of the previous half
    for half in range(HALVES):
        t0 = half * THALF
        x_half = xpool.tile([P, KO, THALF], F32R, tag="x")
        for ko in range(KO):
            eng = nc.scalar if (half == 1 and ko % 2 == 1) else nc.sync
            eng.dma_start(out=x_half[:, ko, :], in_=xv[:, ko, t0:t0 + THALF])
        inter = ipool.tile([P, IO, THALF], F32R, tag="inter")

        # Stage 1: guT = W1^T @ xT, then interT = silu(gateT) * upT
        for jj in range(IO):
            wg = wgpool.tile([P, KO, P], F32R, tag="wg")
            nc.scalar.dma_start(out=wg, in_=w1gv[jj].rearrange("p (ko j) -> p ko j", ko=KO))
            wu = wupool.tile([P, KO, P], F32R, tag="wu")
            nc.scalar.dma_start(out=wu, in_=w1uv[jj].rearrange("p (ko j) -> p ko j", ko=KO))
            pg = [psum1.tile([P, NT], F32, tag="gu", name=f"pg{jj}_{i}") for i in range(TT)]
            pu = [psum1.tile([P, NT], F32, tag="gu", name=f"pu{jj}_{i}") for i in range(TT)]
            for ko in range(KO):
                lw_g = wg[:, ko, :]
                lw_u = wu[:, ko, :]
                for tt in range(TT):
                    mm = nc.tensor.matmul(
                        pg[tt], lw_g,
                        x_half[:, ko, tt * NT:(tt + 1) * NT],
                        start=(ko == 0), stop=(ko == KO - 1),
                    )
                    # Keep the in-order PE on the previous half's stage-2
                    # work instead of parking on an x_half chunk wait.
                    if ko == 0 and jj < 2 and last_s2_mm is not None:
                        _add_dep_helper(mm.ins, last_s2_mm.ins, sync=False,
                                        reason="h2 stage1 after h1 stage2")
                for tt in range(TT):
                    mm = nc.tensor.matmul(
                        pu[tt], lw_u,
                        x_half[:, ko, tt * NT:(tt + 1) * NT],
                        start=(ko == 0), stop=(ko == KO - 1),
                    )
                    if ko == 0 and jj < 2 and last_s2_mm is not None:
                        _add_dep_helper(mm.ins, last_s2_mm.ins, sync=False,
                                        reason="h2 stage1 after h1 stage2")
            for tt in range(TT):
                tmp = tmppool.tile([P, NT], F32, tag="tmp")
                nc.scalar.activation(
                    out=tmp, in_=pg[tt],
                    func=mybir.ActivationFunctionType.Silu,
                )
                nc.vector.tensor_mul(
                    out=inter[:, jj, tt * NT:(tt + 1) * NT],
                    in0=tmp, in1=pu[tt],
                )

        # Stage 2: outT = W2^T @ interT
        for hh in range(HO):
            w2t = w2pool.tile([P, IO, P], F32R, tag="w2")
            nc.scalar.dma_start(out=w2t, in_=w2v[hh].rearrange("p (io j) -> p io j", io=IO))
            ot = opool.tile([P, TT, NT], F32, tag="ot")
            for tt in range(TT):
                po = psum2.tile([P, NT], F32, tag="po")
                for io in range(IO):
                    last_s2_mm = nc.tensor.matmul(
                        po, w2t[:, io, :],
                        inter[:, io, tt * NT:(tt + 1) * NT],
                        start=(io == 0), stop=(io == IO - 1),
                    )
                nc.vector.tensor_copy(out=ot[:, tt, :], in_=po)
                if half == 1 and hh == HO - 1:
                    # tail: stream each tt slice immediately so the final
                    # drain isn't gated on one full 512KB store
                    nc.scalar.dma_start(
                        out=ov[:, hh, t0 + tt * NT:t0 + (tt + 1) * NT],
                        in_=ot[:, tt, :],
                    )
            if not (half == 1 and hh == HO - 1):
                oeng = nc.scalar if half == 1 else nc.gpsimd
                oeng.dma_start(
                    out=ov[:, hh, t0:t0 + THALF], in_=ot,
                )


_NC = None


def _get_nc():
    global _NC
    if _NC is None:
        from contextlib import ExitStack

        nc = bacc.Bacc("TRN2", target_bir_lowering=False, debug=False,
                       num_devices=E)
        xT = nc.dram_tensor("xT", [H, TPC], F32, kind="ExternalInput").ap()
        w1g = nc.dram_tensor("w1g", [IO, P, KO * P], F32,
                             kind="ExternalInput").ap()
        w1u = nc.dram_tensor("w1u", [IO, P, KO * P], F32,
                             kind="ExternalInput").ap()
        w2 = nc.dram_tensor("w2", [HO, P, IO * P], F32,
                            kind="ExternalInput").ap()
        outT = nc.dram_tensor("outT", [H, TPC], F32, kind="ExternalOutput").ap()
        with tile.TileContext(nc) as tc:
            with ExitStack() as ctx:
                _build_mlp(nc, tc, xT, w1g, w1u, w2, outT, ctx)
        nc.compile()
        # Prime the PJRT executable and device state (DGE tables, HAM,
        # HBM) with one throwaway execution so the caller's first real
        # run doesn't pay cold-start costs.
        zero_maps = [
            {
                "xT": np.zeros((H, TPC), np.float32),
                "w1g": np.zeros((IO, P, KO * P), np.float32),
                "w1u": np.zeros((IO, P, KO * P), np.float32),
                "w2": np.zeros((HO, P, IO * P), np.float32),
            }
            for _ in range(E)
        ]
        try:
            run_bass_kernel_spmd(nc, zero_maps, core_ids=list(range(E)))
        except Exception:
            pass
        _NC = nc
    return _NC


def _tile_w1(w1e, col0):
    # w1e: [H, 2*EI]; columns col0:col0+EI tiled to [IO, 128, KO*128]
    # out[jj, p, ko*128 + j] = w1e[ko*128 + p, col0 + jj*128 + j]
    w = w1e[:, col0:col0 + EI].reshape(KO, P, IO, P)
    return np.ascontiguousarray(w.transpose(2, 1, 0, 3).reshape(IO, P, KO * P))


def _tile_w2(w2e):
    # w2e: [EI, H] -> [HO, 128, IO*128]
    # out[hh, p, io*128 + j] = w2e[io*128 + p, hh*128 + j]
    w = w2e.reshape(IO, P, HO, P)
    return np.ascontiguousarray(w.transpose(2, 1, 0, 3).reshape(HO, P, IO * P))


def kernel(x, position_ids, gate_up_proj, down_proj, _trace=False):
    x = np.ascontiguousarray(np.asarray(x, dtype=np.float32))
    B, N, Hd = x.shape
    assert Hd == H
    pid = np.asarray(position_ids)
    if pid.ndim == 1:
        pid = np.broadcast_to(pid[None, :], (B, N))
    pid = pid.reshape(-1).astype(np.int64)
    T = B * N
    eids = pid % E
    order = np.argsort(eids, kind="stable")
    counts = np.bincount(eids, minlength=E)
    assert (counts == T // E).all(), f"unbalanced routing: {counts}"

    flat = x.reshape(T, H)
    xg = flat[order].reshape(E, T // E, H)  # [E, tokens, H] grouped by expert

    gup = np.asarray(gate_up_proj, dtype=np.float32)
    dwn = np.asarray(down_proj, dtype=np.float32)

    in_maps = [
        {
            "xT": np.ascontiguousarray(xg[e].T),
            "w1g": _tile_w1(gup[e], 0),
            "w1u": _tile_w1(gup[e], EI),
            "w2": _tile_w2(dwn[e]),
        }
        for e in range(E)
    ]

    nc = _get_nc()
    res = run_bass_kernel_spmd(nc, in_maps, core_ids=list(range(E)),
                               trace=_trace)

    outg = np.empty((E, T // E, H), dtype=np.float32)
    for e in range(E):
        outg[e] = res.results[e]["outT"].T
    out = np.empty((T, H), dtype=np.float32)
    out[order] = outg.reshape(T, H)
    if _trace:
        kernel.last_exec_time_ns = res.exec_time_ns
        kernel.last_results = res
    return out.reshape(B, N, Hd)


# revision 28
# speedup vs baseline: 1.1211x; 1.1211x over previous
"""Position-routed MLP (expert = position % 8) on 8 NeuronCores.

Expert-parallel: core e runs expert e's dense MLP on its 2048 tokens.
Host gathers tokens by expert (stable sort, same as the reference),
transposes activations to [feature, token] so every matmul's
contraction dim sits on SBUF partitions with no on-device transposes:

    guT[j, t]  = sum_h W1[h, j] * xT[h, t]      (fp32r matmuls)
    interT     = silu(gateT) * upT              (ACT + DVE)
    outT[o, t] = sum_i W2[i, o] * interT[i, t]  (fp32r matmuls)

Tokens are processed in 2 halves of 1024 so x-half + inter + streamed
weight tiles fit in SBUF; weights are re-read once per half.

Weights are pre-tiled on the host ([tile, partition, free] contiguous)
so each weight tile is a single full-bandwidth DMA. Weight loads go on
the scalar engine's HWDGE ring, x loads on sync's, stores on gpsimd
(SWDGE) to avoid FIFO head-of-line blocking between streams.
"""

import numpy as np

import concourse.bass as bass
import concourse.tile as tile
from concourse.bass import _add_dep_helper
from concourse import bacc, mybir
from concourse.bass_utils import run_bass_kernel_spmd

E = 8
H = 2048
EI = 1024
TPC = 4 * 4096 // E  # tokens per core = 2048
P = 128
KO = H // P    # 16 contraction subtiles for stage 1
IO = EI // P   # 8 contraction subtiles for stage 2
HO = H // P    # 16 output-row tiles for stage 2
NT = 512       # moving free dim per matmul (fp32 max)
HALVES = 2
THALF = TPC // HALVES  # 1024
TT = THALF // NT       # 2

F32 = mybir.dt.float32
F32R = mybir.dt.float32r


def _build_mlp(nc: bass.Bass, tc: tile.TileContext, xT, w1g, w1u, w2, outT, ctx):
    # fp32r tags everywhere a tensor feeds a matmul: the BIR verifier
    # requires producers of fp32r-matmul operands to be fp32r themselves.
    xv = xT.bitcast(F32R).rearrange("(ko p) t -> p ko t", p=P)   # [128, 16, 2048]
    w1gv = w1g.bitcast(F32R)  # [IO, 128, KO*128] pre-tiled on host
    w1uv = w1u.bitcast(F32R)  # [IO, 128, KO*128]
    w2v = w2.bitcast(F32R)    # [HO, 128, IO*128]
    ov = outT.rearrange("(ho p) t -> p ho t", p=P)               # [128, 16, 2048]

    xpool = ctx.enter_context(tc.tile_pool(name="x", bufs=1))
    ipool = ctx.enter_context(tc.tile_pool(name="inter", bufs=1))
    wgpool = ctx.enter_context(tc.tile_pool(name="wg", bufs=2))
    wupool = ctx.enter_context(tc.tile_pool(name="wu", bufs=2))
    w2pool = ctx.enter_context(tc.tile_pool(name="w2", bufs=7))
    tmppool = ctx.enter_context(tc.tile_pool(name="tmp", bufs=3))
    opool = ctx.enter_context(tc.tile_pool(name="ostage", bufs=3))
    psum1 = ctx.enter_context(tc.tile_pool(name="psum1", bufs=6, space="PSUM"))
    psum2 = ctx.enter_context(tc.tile_pool(name="psum2", bufs=2, space="PSUM"))

    last_s2_mm = None  # last stage-2 matmul of the previous half
    for half in range(HALVES):
        t0 = half * THALF
        x_half = xpool.tile([P, KO, THALF], F32R, tag="x")
        for ko in range(KO):
            eng = nc.gpsimd if (half == 1 and ko % 2 == 1) else nc.sync
            eng.dma_start(out=x_half[:, ko, :], in_=xv[:, ko, t0:t0 + THALF])
        inter = ipool.tile([P, IO, THALF], F32R, tag="inter")

        # Stage 1: guT = W1^T @ xT, then interT = silu(gateT) * upT
        for jj in range(IO):
            wg = wgpool.tile([P, KO, P], F32R, tag="wg")
            nc.scalar.dma_start(out=wg, in_=w1gv[jj].rearrange("p (ko j) -> p ko j", ko=KO))
            wu = wupool.tile([P, KO, P], F32R, tag="wu")
            nc.scalar.dma_start(out=wu, in_=w1uv[jj].rearrange("p (ko j) -> p ko j", ko=KO))
            pg = [psum1.tile([P, NT], F32, tag="gu", name=f"pg{jj}_{i}") for i in range(TT)]
            pu = [psum1.tile([P, NT], F32, tag="gu", name=f"pu{jj}_{i}") for i in range(TT)]
            for ko in range(KO):
                lw_g = wg[:, ko, :]
                lw_u = wu[:, ko, :]
                for tt in range(TT):
                    mm = nc.tensor.matmul(
                        pg[tt], lw_g,
                        x_half[:, ko, tt * NT:(tt + 1) * NT],
                        start=(ko == 0), stop=(ko == KO - 1),
                    )
                    # Keep the in-order PE on the previous half's stage-2
                    # work instead of parking on an x_half chunk wait.
                    if ko == 0 and jj < 2 and last_s2_mm is not None:
                        _add_dep_helper(mm.ins, last_s2_mm.ins, sync=False,
                                        reason="h2 stage1 after h1 stage2")
                for tt in range(TT):
                    mm = nc.tensor.matmul(
                        pu[tt], lw_u,
                        x_half[:, ko, tt * NT:(tt + 1) * NT],
                        start=(ko == 0), stop=(ko == KO - 1),
                    )
                    if ko == 0 and jj < 2 and last_s2_mm is not None:
                        _add_dep_helper(mm.ins, last_s2_mm.ins, sync=False,
                                        reason="h2 stage1 after h1 stage2")
            for tt in range(TT):
                tmp = tmppool.tile([P, NT], F32, tag="tmp")
                nc.scalar.activation(
                    out=tmp, in_=pg[tt],
                    func=mybir.ActivationFunctionType.Silu,
                )
                nc.vector.tensor_mul(
                    out=inter[:, jj, tt * NT:(tt + 1) * NT],
                    in0=tmp, in1=pu[tt],
                )

        # Stage 2: outT = W2^T @ interT
        for hh in range(HO):
            w2t = w2pool.tile([P, IO, P], F32R, tag="w2")
            nc.scalar.dma_start(out=w2t, in_=w2v[hh].rearrange("p (io j) -> p io j", io=IO))
            ot = opool.tile([P, TT, NT], F32, tag="ot")
            for tt in range(TT):
                po = psum2.tile([P, NT], F32, tag="po")
                for io in range(IO):
                    last_s2_mm = nc.tensor.matmul(
                        po, w2t[:, io, :],
                        inter[:, io, tt * NT:(tt + 1) * NT],
                        start=(io == 0), stop=(io == IO - 1),
                    )
                nc.vector.tensor_copy(out=ot[:, tt, :], in_=po)
                if half == 1 and hh == HO - 1:
                    # tail: stream each tt slice immediately so the final
                    # drain isn't gated on one full 512KB store
                    nc.scalar.dma_start(
                        out=ov[:, hh, t0 + tt * NT:t0 + (tt + 1) * NT],
                        in_=ot[:, tt, :],
                    )
            if not (half == 1 and hh == HO - 1):
                oeng = nc.scalar if half == 1 else nc.gpsimd
                oeng.dma_start(
                    out=ov[:, hh, t0:t0 + THALF], in_=ot,
                )


_NC = None


def _get_nc():
    global _NC
    if _NC is None:
        from contextlib import ExitStack

        nc = bacc.Bacc("TRN2", target_bir_lowering=False, debug=False,
                       num_devices=E)
        xT = nc.dram_tensor("xT", [H, TPC], F32, kind="ExternalInput").ap()
        w1g = nc.dram_tensor("w1g", [IO, P, KO * P], F32,
                             kind="ExternalInput").ap()
        w1u = nc.dram_tensor("w1u", [IO, P, KO * P], F32,
                             kind="ExternalInput").ap()
        w2 = nc.dram_tensor("w2", [HO, P, IO * P], F32,
                            kind="ExternalInput").ap()
        outT = nc.dram_tensor("outT", [H, TPC], F32, kind="ExternalOutput").ap()
        with tile.TileContext(nc) as tc:
            with ExitStack() as ctx:
                _build_mlp(nc, tc, xT, w1g, w1u, w2, outT, ctx)
        nc.compile()
        # Prime the PJRT executable and device state (DGE tables, HAM,
        # HBM) with one throwaway execution so the caller's first real
        # run doesn't pay cold-start costs.
        zero_maps = [
            {
                "xT": np.zeros((H, TPC), np.float32),
                "w1g": np.zeros((IO, P, KO * P), np.float32),
                "w1u": np.zeros((IO, P, KO * P), np.float32),
                "w2": np.zeros((HO, P, IO * P), np.float32),
            }
            for _ in range(E)
        ]
        try:
            run_bass_kernel_spmd(nc, zero_maps, core_ids=list(range(E)))
        except Exception:
            pass
        _NC = nc
    return _NC


def _tile_w1(w1e, col0):
    # w1e: [H, 2*EI]; columns col0:col0+EI tiled to [IO, 128, KO*128]
    # out[jj, p, ko*128 + j] = w1e[ko*128 + p, col0 + jj*128 + j]
    w = w1e[:, col0:col0 + EI].reshape(KO, P, IO, P)
    return np.ascontiguousarray(w.transpose(2, 1, 0, 3).reshape(IO, P, KO * P))


def _tile_w2(w2e):
    # w2e: [EI, H] -> [HO, 128, IO*128]
    # out[hh, p, io*128 + j] = w2e[io*128 + p, hh*128 + j]
    w = w2e.reshape(IO, P, HO, P)
    return np.ascontiguousarray(w.transpose(2, 1, 0, 3).reshape(HO, P, IO * P))


def kernel(x, position_ids, gate_up_proj, down_proj, _trace=False):
    x = np.ascontiguousarray(np.asarray(x, dtype=np.float32))
    B, N, Hd = x.shape
    assert Hd == H
    pid = np.asarray(position_ids)
    if pid.ndim == 1:
        pid = np.broadcast_to(pid[None, :], (B, N))
    pid = pid.reshape(-1).astype(np.int64)
    T = B * N
    eids = pid % E
    order = np.argsort(eids, kind="stable")
    counts = np.bincount(eids, minlength=E)
    assert (counts == T // E).all(), f"unbalanced routing: {counts}"

    flat = x.reshape(T, H)
    xg = flat[order].reshape(E, T // E, H)  # [E, tokens, H] grouped by expert

    gup = np.asarray(gate_up_proj, dtype=np.float32)
    dwn = np.asarray(down_proj, dtype=np.float32)

    in_maps = [
        {
            "xT": np.ascontiguousarray(xg[e].T),
            "w1g": _tile_w1(gup[e], 0),
            "w1u": _tile_w1(gup[e], EI),
            "w2": _tile_w2(dwn[e]),
        }
        for e in range(E)
    ]

    nc = _get_nc()
    res = run_bass_kernel_spmd(nc, in_maps, core_ids=list(range(E)),
                               trace=_trace)

    outg = np.empty((E, T // E, H), dtype=np.float32)
    for e in range(E):
        outg[e] = res.results[e]["outT"].T
    out = np.empty((T, H), dtype=np.float32)
    out[order] = outg.reshape(T, H)
    if _trace:
        kernel.last_exec_time_ns = res.exec_time_ns
        kernel.last_results = res
    return out.reshape(B, N, Hd)


# revision 30
# speedup vs baseline: 1.1652x; 1.0394x over previous
"""Position-routed MLP (expert = position % 8) on 8 NeuronCores.

Expert-parallel: core e runs expert e's dense MLP on its 2048 tokens.
Host gathers tokens by expert (stable sort, same as the reference),
transposes activations to [feature, token] so every matmul's
contraction dim sits on SBUF partitions with no on-device transposes:

    guT[j, t]  = sum_h W1[h, j] * xT[h, t]      (fp32r matmuls)
    interT     = silu(gateT) * upT              (ACT + DVE)
    outT[o, t] = sum_i W2[i, o] * interT[i, t]  (fp32r matmuls)

Tokens are processed in 2 halves of 1024 so x-half + inter + streamed
weight tiles fit in SBUF; weights are re-read once per half.

Weights are pre-tiled on the host ([tile, partition, free] contiguous)
so each weight tile is a single full-bandwidth DMA. Weight loads go on
the scalar engine's HWDGE ring, x loads on sync's, stores on gpsimd
(SWDGE) to avoid FIFO head-of-line blocking between streams.
"""

import numpy as np

import concourse.bass as bass
import concourse.tile as tile
from concourse.bass import _add_dep_helper
from concourse import bacc, mybir
from concourse.bass_utils import run_bass_kernel_spmd

E = 8
H = 2048
EI = 1024
TPC = 4 * 4096 // E  # tokens per core = 2048
P = 128
KO = H // P    # 16 contraction subtiles for stage 1
IO = EI // P   # 8 contraction subtiles for stage 2
HO = H // P    # 16 output-row tiles for stage 2
NT = 512       # moving free dim per matmul (fp32 max)
HALVES = 2
THALF = TPC // HALVES  # 1024
TT = THALF // NT       # 2

F32 = mybir.dt.float32
F32R = mybir.dt.float32r


def _build_mlp(nc: bass.Bass, tc: tile.TileContext, xT, w1g, w1u, w2, outT, ctx):
    # fp32r tags everywhere a tensor feeds a matmul: the BIR verifier
    # requires producers of fp32r-matmul operands to be fp32r themselves.
    xv = xT.bitcast(F32R).rearrange("(ko p) t -> p ko t", p=P)   # [128, 16, 2048]
    w1gv = w1g.bitcast(F32R)  # [IO, 128, KO*128] pre-tiled on host
    w1uv = w1u.bitcast(F32R)  # [IO, 128, KO*128]
    w2v = w2.bitcast(F32R)    # [HO, 128, IO*128]
    ov = outT.rearrange("(ho p) t -> p ho t", p=P)               # [128, 16, 2048]

    xpool = ctx.enter_context(tc.tile_pool(name="x", bufs=1))
    ipool = ctx.enter_context(tc.tile_pool(name="inter", bufs=1))
    wgpool = ctx.enter_context(tc.tile_pool(name="wg", bufs=2))
    wupool = ctx.enter_context(tc.tile_pool(name="wu", bufs=2))
    w2pool = ctx.enter_context(tc.tile_pool(name="w2", bufs=7))
    tmppool = ctx.enter_context(tc.tile_pool(name="tmp", bufs=3))
    opool = ctx.enter_context(tc.tile_pool(name="ostage", bufs=3))
    psum1 = ctx.enter_context(tc.tile_pool(name="psum1", bufs=6, space="PSUM"))
    psum2 = ctx.enter_context(tc.tile_pool(name="psum2", bufs=2, space="PSUM"))

    last_s2_mm = None  # last stage-2 matmul of the previous half
    for half in range(HALVES):
        t0 = half * THALF
        x_half = xpool.tile([P, KO, THALF], F32R, tag="x")
        for ko in range(KO):
            nc.sync.dma_start(out=x_half[:, ko, :], in_=xv[:, ko, t0:t0 + THALF])
        inter = ipool.tile([P, IO, THALF], F32R, tag="inter")

        # Stage 1: guT = W1^T @ xT, then interT = silu(gateT) * upT
        for jj in range(IO):
            wg = wgpool.tile([P, KO, P], F32R, tag="wg")
            nc.scalar.dma_start(out=wg, in_=w1gv[jj].rearrange("p (ko j) -> p ko j", ko=KO))
            wu = wupool.tile([P, KO, P], F32R, tag="wu")
            nc.scalar.dma_start(out=wu, in_=w1uv[jj].rearrange("p (ko j) -> p ko j", ko=KO))
            pg = [psum1.tile([P, NT], F32, tag="gu", name=f"pg{jj}_{i}") for i in range(TT)]
            pu = [psum1.tile([P, NT], F32, tag="gu", name=f"pu{jj}_{i}") for i in range(TT)]
            for ko in range(KO):
                lw_g = wg[:, ko, :]
                lw_u = wu[:, ko, :]
                for tt in range(TT):
                    mm = nc.tensor.matmul(
                        pg[tt], lw_g,
                        x_half[:, ko, tt * NT:(tt + 1) * NT],
                        start=(ko == 0), stop=(ko == KO - 1),
                    )
                    # Keep the in-order PE on the previous half's stage-2
                    # work instead of parking on an x_half chunk wait.
                    if ko == 0 and jj < 2 and last_s2_mm is not None:
                        _add_dep_helper(mm.ins, last_s2_mm.ins, sync=False,
                                        reason="h2 stage1 after h1 stage2")
                for tt in range(TT):
                    mm = nc.tensor.matmul(
                        pu[tt], lw_u,
                        x_half[:, ko, tt * NT:(tt + 1) * NT],
                        start=(ko == 0), stop=(ko == KO - 1),
                    )
                    if ko == 0 and jj < 2 and last_s2_mm is not None:
                        _add_dep_helper(mm.ins, last_s2_mm.ins, sync=False,
                                        reason="h2 stage1 after h1 stage2")
            for tt in range(TT):
                tmp = tmppool.tile([P, NT], F32, tag="tmp")
                nc.scalar.activation(
                    out=tmp, in_=pg[tt],
                    func=mybir.ActivationFunctionType.Silu,
                )
                nc.vector.tensor_mul(
                    out=inter[:, jj, tt * NT:(tt + 1) * NT],
                    in0=tmp, in1=pu[tt],
                )

        # Stage 2: outT = W2^T @ interT
        for hh in range(HO):
            w2t = w2pool.tile([P, IO, P], F32R, tag="w2")
            nc.scalar.dma_start(out=w2t, in_=w2v[hh].rearrange("p (io j) -> p io j", io=IO))
            ot = opool.tile([P, TT, NT], F32, tag="ot")
            for tt in range(TT):
                po = psum2.tile([P, NT], F32, tag="po")
                for io in range(IO):
                    last_s2_mm = nc.tensor.matmul(
                        po, w2t[:, io, :],
                        inter[:, io, tt * NT:(tt + 1) * NT],
                        start=(io == 0), stop=(io == IO - 1),
                    )
                nc.vector.tensor_copy(out=ot[:, tt, :], in_=po)
                if half == 1 and hh == HO - 1:
                    # tail: stream each tt slice immediately so the final
                    # drain isn't gated on one full 512KB store
                    nc.scalar.dma_start(
                        out=ov[:, hh, t0 + tt * NT:t0 + (tt + 1) * NT],
                        in_=ot[:, tt, :],
                    )
            if not (half == 1 and hh == HO - 1):
                oeng = nc.scalar if half == 1 else nc.gpsimd
                oeng.dma_start(
                    out=ov[:, hh, t0:t0 + THALF], in_=ot,
                )


_NC = None


def _get_nc():
    global _NC
    if _NC is None:
        from contextlib import ExitStack

        nc = bacc.Bacc("TRN2", target_bir_lowering=False, debug=False,
                       num_devices=E)
        xT = nc.dram_tensor("xT", [H, TPC], F32, kind="ExternalInput").ap()
        w1g = nc.dram_tensor("w1g", [IO, P, KO * P], F32,
                             kind="ExternalInput").ap()
        w1u = nc.dram_tensor("w1u", [IO, P, KO * P], F32,
                             kind="ExternalInput").ap()
        w2 = nc.dram_tensor("w2", [HO, P, IO * P], F32,
                            kind="ExternalInput").ap()
        outT = nc.dram_tensor("outT", [H, TPC], F32, kind="ExternalOutput").ap()
        with tile.TileContext(nc) as tc:
            with ExitStack() as ctx:
                _build_mlp(nc, tc, xT, w1g, w1u, w2, outT, ctx)
        nc.compile()
        # Prime the PJRT executable and device state (DGE tables, HAM,
        # HBM) with one throwaway execution so the caller's first real
        # run doesn't pay cold-start costs.
        zero_maps = [
            {
                "xT": np.zeros((H, TPC), np.float32),
                "w1g": np.zeros((IO, P, KO * P), np.float32),
                "w1u": np.zeros((IO, P, KO * P), np.float32),
                "w2": np.zeros((HO, P, IO * P), np.float32),
            }
            for _ in range(E)
        ]
        try:
            run_bass_kernel_spmd(nc, zero_maps, core_ids=list(range(E)))
        except Exception:
            pass
        _NC = nc
    return _NC


def _tile_w1(w1e, col0):
    # w1e: [H, 2*EI]; columns col0:col0+EI tiled to [IO, 128, KO*128]
    # out[jj, p, ko*128 + j] = w1e[ko*128 + p, col0 + jj*128 + j]
    w = w1e[:, col0:col0 + EI].reshape(KO, P, IO, P)
    return np.ascontiguousarray(w.transpose(2, 1, 0, 3).reshape(IO, P, KO * P))


def _tile_w2(w2e):
    # w2e: [EI, H] -> [HO, 128, IO*128]
    # out[hh, p, io*128 + j] = w2e[io*128 + p, hh*128 + j]
    w = w2e.reshape(IO, P, HO, P)
    return np.ascontiguousarray(w.transpose(2, 1, 0, 3).reshape(HO, P, IO * P))


def kernel(x, position_ids, gate_up_proj, down_proj, _trace=False):
    x = np.ascontiguousarray(np.asarray(x, dtype=np.float32))
    B, N, Hd = x.shape
    assert Hd == H
    pid = np.asarray(position_ids)
    if pid.ndim == 1:
        pid = np.broadcast_to(pid[None, :], (B, N))
    pid = pid.reshape(-1).astype(np.int64)
    T = B * N
    eids = pid % E
    order = np.argsort(eids, kind="stable")
    counts = np.bincount(eids, minlength=E)
    assert (counts == T // E).all(), f"unbalanced routing: {counts}"

    flat = x.reshape(T, H)
    xg = flat[order].reshape(E, T // E, H)  # [E, tokens, H] grouped by expert

    gup = np.asarray(gate_up_proj, dtype=np.float32)
    dwn = np.asarray(down_proj, dtype=np.float32)

    in_maps = [
        {
            "xT": np.ascontiguousarray(xg[e].T),
            "w1g": _tile_w1(gup[e], 0),
            "w1u": _tile_w1(gup[e], EI),
            "w2": _tile_w2(dwn[e]),
        }
        for e in range(E)
    ]

    nc = _get_nc()
    res = run_bass_kernel_spmd(nc, in_maps, core_ids=list(range(E)),
                               trace=_trace)

    outg = np.empty((E, T // E, H), dtype=np.float32)
    for e in range(E):
        outg[e] = res.results[e]["outT"].T
    out = np.empty((T, H), dtype=np.float32)
    out[order] = outg.reshape(T, H)
    if _trace:
        kernel.last_exec_time_ns = res.exec_time_ns
        kernel.last_results = res
    return out.reshape(B, N, Hd)
